# revision 1
# baseline (speedup 1.0000x reference)
"""Trainium2 Bass kernel for causal multi-head attention + output projection.

Problem (hardcoded): x[4, 2048, 1024] fp32, 16 heads, head_dim 64, causal,
torch-Linear convention (y = x @ W.T), output projection with bias.

Sharding over 8 NeuronCores: batch (4) x head-group (2 groups of 8 heads).
Each core computes q/k/v for its 8 heads of its batch, causal attention in
the S^T layout (keys on partitions, queries on free dim; softmax denominators
produced by an appended ones-column in V), then the output projection.

Combine modes:
  - "a2a": on-device AllToAll per head swaps query-halves between the two
    cores of a batch so each core projects all 16 heads for its own 1024
    queries; outputs are disjoint rows, host just concatenates.
  - "hostsum": each core emits a partial projection over its 8 heads for all
    2048 queries; host sums the pair (bias folded into group-0's input).

All matmuls run as float32r (TF32-like, ~1.5e-4 rel err, 4x faster than fp32).
"""
import os
import sys
import types

import numpy as np

import concourse.bass as bass
import concourse.mybir as mybir
import concourse.tile as tile
from concourse import bacc, bass_utils

DT = getattr(mybir.dt, os.environ.get("ATTN_DT_MAIN", "float32r"))
F32 = mybir.dt.float32
AF = mybir.ActivationFunctionType
OP = mybir.AluOpType

B, T, D = 4, 2048, 1024
H, HD = 16, 64
HG = 8          # heads per core
QH = T // 2     # query half
N_CORES = 8
SCALE = 1.0 / 8.0

MODE = os.environ.get("ATTN_KERNEL_MODE", "ag")
ADT_NAME = os.environ.get("ATTN_DTYPE", "float32r")
ADT = getattr(mybir.dt, ADT_NAME)


# ---------------------------------------------------------------------------
# environment glue
# ---------------------------------------------------------------------------

def _install_ntff_hook():
    if 'antenv.axon_hooks' in sys.modules:
        return
    try:
        from trn_agent_boot.trn_boot import _ntff_profile_via_ctypes
        hook = _ntff_profile_via_ctypes('/opt/axon/libaxon_pjrt.so')
    except Exception:
        hook = None
    mod = types.ModuleType('antenv.axon_hooks')
    mod.get_axon_ntff_profile_hook = lambda: hook
    mod.set_axon_ntff_profile_hook = lambda h: None
    sys.modules['antenv.axon_hooks'] = mod


def _run_spmd(nc, in_maps, trace=False):
    from concourse.bass_interp import get_hw_module
    bass_utils.upload_artifacts = lambda tmpdir: tmpdir
    if trace:
        _install_ntff_hook()
    old_m = nc.m
    nc.m = get_hw_module(nc.m)
    try:
        return bass_utils.run_bass_kernel_spmd(
            nc, in_maps, core_ids=list(range(N_CORES)),
            trace=trace, trace_cores=[0] if trace else None,
        )
    finally:
        nc.m = old_m


# ---------------------------------------------------------------------------
# kernel program
# ---------------------------------------------------------------------------

def _qkv_phase(nc, tc, ctx, xT, wqT, wkT, wvT, vone, qT_sb, kT_sb, v_sb):
    """Compute q.T [512,2048], k.T [512,2048] and V' [2048, 8, 65] for this
    core's 8 heads. Contraction dim D lives on partitions; all operands fp32r."""
    xp = ctx.enter_context(tc.tile_pool(name="xph", bufs=16))
    wp = ctx.enter_context(tc.tile_pool(name="wph", bufs=12))
    ps = ctx.enter_context(tc.tile_pool(name="p2ps", bufs=2, space="PSUM"))

    xT_r = xT.rearrange("(ko ki) t -> ki ko t", ki=128)

    def load_w(wT):
        parts = []
        wT_r = wT.rearrange("(ko ki) n -> ki ko n", ki=128)
        for kk in range(8):
            t = wp.tile([128, 512], DT, tag="w")
            nc.sync.dma_start(t[:], wT_r[:, kk])
            parts.append(t)
        return parts

    # k.T first: emit its weight slices and the x slices in consumption order
    # so the first matmuls start as soon as ~1MB has landed.
    wk_sb = load_w(wkT)
    xh = [[None] * 8 for _ in range(2)]
    for kk in range(8):
        for half in range(2):
            t = xp.tile([128, QH], DT, tag="xh")
            nc.sync.dma_start(t[:], xT_r[:, kk, half * QH:(half + 1) * QH])
            xh[half][kk] = t
    for m in range(4):
        for half in range(2):
            pt = ps.tile([128, QH], F32, tag="st")
            for nch in range(2):
                sl = slice(nch * 512, (nch + 1) * 512)
                for kk in range(8):
                    nc.tensor.matmul(
                        pt[:, sl],
                        lhsT=wk_sb[kk][:, m * 128:(m + 1) * 128],
                        rhs=xh[half][kk][:, sl],
                        start=(kk == 0), stop=(kk == 7))
            nc.vector.tensor_copy(kT_sb[:, m, half * QH:(half + 1) * QH], pt[:])

    wv_sb = load_w(wvT)
    nc.sync.dma_start(v_sb[:, :, :, 64],
                      vone.rearrange("p (a b) -> p a b", a=16))
    for m in range(16):
        pt = ps.tile([128, QH], F32, tag="st")
        for kk in range(8):
            nc.tensor.matmul(
                pt[:, 0:512],
                lhsT=xh[m // 8][kk][:, (m % 8) * 128:(m % 8 + 1) * 128],
                rhs=wv_sb[kk][:],
                start=(kk == 0), stop=(kk == 7))
        nc.vector.tensor_copy(
            v_sb[:, m, :, 0:64],
            pt[:, 0:512].rearrange("p (h d) -> p h d", h=HG))

    wq_sb = load_w(wqT)
    for half in range(2):
        for m in range(4):
            pt = ps.tile([128, QH], F32, tag="st")
            for nch in range(2):
                sl = slice(nch * 512, (nch + 1) * 512)
                for kk in range(8):
                    nc.tensor.matmul(
                        pt[:, sl],
                        lhsT=wq_sb[kk][:, m * 128:(m + 1) * 128],
                        rhs=xh[half][kk][:, sl],
                        start=(kk == 0), stop=(kk == 7))
            nc.vector.tensor_copy(qT_sb[:, m, half * QH:(half + 1) * QH], pt[:])


def _attend_pair(nc, p, qT_sb, kT_sb, v_sb, mask_sb, ps, es, snum, srec,
                 evict_cb, norm_cb):
    """Heads (2p, 2p+1) with their j-steps interleaved so the PE always has an
    independent S/AV matmul while the other head's exp runs on ACT. Rows 0..63
    of each accumulator are un-normalized O.T, row 64 the softmax denominators;
    normalization uses a reciprocal spread over 64 partitions via DRAM."""
    heads = (2 * p, 2 * p + 1)
    for qh in range(2):
        jmax = 8 * qh + 8
        o_ps = {h: ps.tile([65, QH], F32, tag="o", name=f"o{h}_{qh}")
                for h in heads}
        for j in range(jmax):
            qstart = max(QH * qh, 128 * j)
            n = QH * (qh + 1) - qstart
            coff = qstart - QH * qh
            e_sbs = {}
            for h in heads:
                pbase = 64 * (h % 2)
                sub = h // 2
                s_ps = ps.tile([128, QH], F32, tag="st", name=f"s{h}")
                for c in range(0, n, 512):
                    cn = min(512, n - c)
                    nc.tensor.matmul(
                        s_ps[:, c:c + cn],
                        lhsT=kT_sb[pbase:pbase + 64, sub, j * 128:(j + 1) * 128],
                        rhs=qT_sb[pbase:pbase + 64, sub,
                                  qstart + c:qstart + c + cn],
                        start=True, stop=True)
                e_sb = es.tile([128, QH], ADT, tag="es", name=f"e{h}")
                nc.scalar.activation(e_sb[:, 0:n], s_ps[:, 0:n], AF.Exp,
                                     scale=SCALE)
                if j >= 8 * qh:
                    nc.vector.tensor_tensor(
                        e_sb[:, 0:128], e_sb[:, 0:128], mask_sb[:], OP.mult)
                e_sbs[h] = e_sb
            for h in heads:
                c0 = coff
                while c0 < QH:
                    hi = min(QH, (c0 // 512 + 1) * 512)
                    nc.tensor.matmul(
                        o_ps[h][:, c0:hi],
                        lhsT=v_sb[:, j, h, :],
                        rhs=e_sbs[h][:, c0 - coff:hi - coff],
                        start=(j == 0), stop=(j == jmax - 1),
                        skip_group_check=True)
                    c0 = hi
        for h in heads:
            evict_cb(h, qh, o_ps[h])
            i = 4 * (h // 2) + 2 * (h % 2) + qh
            stmp = es.tile([1, QH], F32, tag="sr")
            nc.scalar.copy(stmp[:], o_ps[h][64:65, :])
            nc.sync.dma_start(snum[i:i + 1, :], stmp[:])
            st64 = es.tile([64, QH // 64], F32, tag="sp")
            nc.sync.dma_start(st64[:], snum[i].rearrange("(p f) -> p f", p=64))
            nc.vector.reciprocal(st64[:], st64[:])
            nc.sync.dma_start(srec[i].rearrange("(p f) -> p f", p=64), st64[:])
            bc = es.tile([128, QH], F32, tag="bc")
            nc.sync.dma_start(bc[:], srec[i][None, :].broadcast_to([128, QH]))
            norm_cb(h, qh, bc)


def build_nc(mode):
    nc = bacc.Bacc("TRN2", target_bir_lowering=False, debug=False,
                   enable_asserts=False, num_devices=N_CORES)
    xT = nc.dram_tensor("xT", [D, T], DT, kind="ExternalInput").ap()
    wqT = nc.dram_tensor("wqT", [D, 512], DT, kind="ExternalInput").ap()
    wkT = nc.dram_tensor("wkT", [D, 512], DT, kind="ExternalInput").ap()
    wvT = nc.dram_tensor("wvT", [D, 512], DT, kind="ExternalInput").ap()
    mask = nc.dram_tensor("mask", [128, 128], ADT, kind="ExternalInput").ap()
    vone = nc.dram_tensor("vone", [128, 128], ADT, kind="ExternalInput").ap()
    snum = nc.dram_tensor("snum", [16, QH], F32).ap()
    srec = nc.dram_tensor("srec", [16, QH], F32).ap()
    agin = agout = None
    if mode == "ag":
        agin = [nc.dram_tensor(f"agin{h}", [64, T], DT).ap() for h in range(HG)]
        agout = [nc.dram_tensor(f"agout{h}", [128, T], DT).ap()
                 for h in range(HG)]
    if mode == "ag":
        wpT = nc.dram_tensor("wpT", [D, D], DT, kind="ExternalInput").ap()
        bias = nc.dram_tensor("bias", [1, D], F32, kind="ExternalInput").ap()
        y = nc.dram_tensor("y", [T, D], F32, kind="ExternalOutput").ap()
    else:
        wpT = nc.dram_tensor("wpT", [512, D], DT, kind="ExternalInput").ap()
        bias = nc.dram_tensor("bias", [1, D], F32, kind="ExternalInput").ap()
        y = nc.dram_tensor("y", [T, D], F32, kind="ExternalOutput").ap()

    from contextlib import ExitStack
    with tile.TileContext(nc) as tc, ExitStack() as ctx:
        per = ctx.enter_context(tc.tile_pool(name="per", bufs=1))

        qT_sb = per.tile([128, 4, T], ADT, tag="qT")
        kT_sb = per.tile([128, 4, T], ADT, tag="kT")
        v_sb = per.tile([128, 16, HG, 65], ADT, tag="v")
        mask_sb = per.tile([128, 128], ADT, tag="mask")

        nc.sync.dma_start(mask_sb[:], mask[:])
        bias_f = bias[0]

        with ExitStack() as p2:
            _qkv_phase(nc, tc, p2, xT, wqT, wkT, wvT, vone, qT_sb, kT_sb, v_sb)

        # O accumulator (lives from attention through projection)
        mid = ctx.enter_context(tc.tile_pool(name="mid", bufs=1))
        o_all = None
        if mode != "ag":
            o_all = mid.tile([128, 4, T], DT, tag="oacc")
        wp_sb = mid.tile([128, 8 if mode == "ag" else 4, D], DT, tag="wp")
        nc.sync.dma_start(wp_sb[:],
                          wpT.rearrange("(ko ki) n -> ki ko n", ki=128))

        with ExitStack() as attn:
            ps = attn.enter_context(tc.tile_pool(name="aps", bufs=2, space="PSUM"))
            es = attn.enter_context(tc.tile_pool(name="es", bufs=3))

            if mode == "ag":
                oh_pool = attn.enter_context(tc.tile_pool(name="oh", bufs=2))
                for p in range(HG // 2):
                    ohs = {h: oh_pool.tile([64, T], DT, tag="oh", name=f"oh{h}")
                           for h in (2 * p, 2 * p + 1)}

                    def evict_cb(h, qh, o_ps, ohs=ohs):
                        nc.vector.tensor_copy(
                            ohs[h][:, QH * qh:QH * (qh + 1)], o_ps[0:64, :])

                    def norm_cb(h, qh, bc, ohs=ohs):
                        sl_ap = ohs[h][:, QH * qh:QH * (qh + 1)]
                        nc.vector.tensor_tensor(sl_ap, sl_ap, bc[0:64, :],
                                                OP.mult)

                    _attend_pair(nc, p, qT_sb, kT_sb, v_sb, mask_sb,
                                 ps, es, snum, srec, evict_cb, norm_cb)

                    for h in (2 * p, 2 * p + 1):
                        nc.sync.dma_start(agin[h][:], ohs[h][:])
                        nc.gpsimd.collective_compute(
                            "AllGather", OP.bypass,
                            replica_groups=[[0, 1], [2, 3], [4, 5], [6, 7]],
                            ins=[agin[h][:]], outs=[agout[h][:]],
                        )
            else:
                def evict_cb(h, qh, o_ps):
                    nc.vector.tensor_copy(
                        o_all[64 * (h % 2):64 * (h % 2) + 64, h // 2,
                              QH * qh:QH * (qh + 1)],
                        o_ps[0:64, :])

                def norm_cb(h, qh, bc):
                    pb = 64 * (h % 2)
                    sl_ap = o_all[pb:pb + 64, h // 2, QH * qh:QH * (qh + 1)]
                    nc.vector.tensor_tensor(sl_ap, sl_ap, bc[pb:pb + 64, :],
                                            OP.mult)

                for p in range(HG // 2):
                    _attend_pair(nc, p, qT_sb, kT_sb, v_sb, mask_sb,
                                 ps, es, snum, srec, evict_cb, norm_cb)

        # projection (psum from the attention pool so the scheduler can
        # overlap early m-tiles with the last head's attention)
        n_kk = 8 if mode == "ag" else 4
        bias_bc = mid.tile([128, D], F32, tag="bbc")
        nc.sync.dma_start(bias_bc[:], bias_f[None, :].broadcast_to([128, D]))
        yo = ctx.enter_context(tc.tile_pool(name="yo", bufs=3))
        ost = ctx.enter_context(tc.tile_pool(name="ost", bufs=10))
        pps = ctx.enter_context(tc.tile_pool(name="pps", bufs=2, space="PSUM"))
        for m in range(16):
            lts = []
            if mode == "ag":
                for kk in range(n_kk):
                    lt = ost.tile([128, 128], DT, tag="ol")
                    for r in range(2):
                        gh = 2 * kk + r
                        nc.sync.dma_start(
                            lt[64 * r:64 * r + 64, :],
                            agout[gh % 8][64 * (gh // 8):64 * (gh // 8) + 64,
                                          m * 128:(m + 1) * 128])
                    lts.append(lt)
            yp = pps.tile([128, D], F32, tag="yp")
            for nch in range(2):
                sl = slice(nch * 512, (nch + 1) * 512)
                for kk in range(n_kk):
                    lhsT = (lts[kk][:] if mode == "ag"
                            else o_all[:, kk, m * 128:(m + 1) * 128])
                    nc.tensor.matmul(
                        yp[:, sl], lhsT=lhsT, rhs=wp_sb[:, kk, sl],
                        start=(kk == 0), stop=(kk == n_kk - 1))
            y_sb = yo.tile([128, D], F32, tag="y")
            nc.vector.tensor_tensor(y_sb[:], yp[:], bias_bc[:], OP.add)
            nc.sync.dma_start(y[m * 128:(m + 1) * 128, :], y_sb[:])

    nc.compile()
    return nc


# ---------------------------------------------------------------------------
# host-side sharding + entry point
# ---------------------------------------------------------------------------

_NC_CACHE = {}


def _get_nc(mode):
    if mode not in _NC_CACHE:
        _NC_CACHE[mode] = build_nc(mode)
    return _NC_CACHE[mode]


def _make_in_maps(x, Wq, Wk, Wv, Wp, bp, mode):
    x = np.asarray(x, dtype=np.float32)
    Wq = np.asarray(Wq, dtype=np.float32)
    Wk = np.asarray(Wk, dtype=np.float32)
    Wv = np.asarray(Wv, dtype=np.float32)
    Wp = np.asarray(Wp, dtype=np.float32)
    bp = np.asarray(bp, dtype=np.float32)

    adt_np = mybir.dt.np(ADT)
    dt_np = mybir.dt.np(DT)
    mask = np.zeros((128, 128), dtype=np.float32)
    k_idx = np.arange(128)[:, None]
    q_idx = np.arange(128)[None, :]
    mask[q_idx >= k_idx] = 1.0
    mask = mask.astype(adt_np)

    xTs = [np.ascontiguousarray(x[b].T) for b in range(B)]
    in_maps = []
    for c in range(N_CORES):
        b, g = c // 2, c % 2
        rows = slice(512 * g, 512 * (g + 1))
        m = {
            "xT": xTs[b].astype(dt_np),
            "wqT": np.ascontiguousarray(Wq[rows, :].T).astype(dt_np),
            "wkT": np.ascontiguousarray(Wk[rows, :].T).astype(dt_np),
            "wvT": np.ascontiguousarray(Wv[rows, :].T).astype(dt_np),
            "mask": mask,
            "vone": np.ones((128, 128), dtype=adt_np),
        }
        if mode == "ag":
            m["wpT"] = np.ascontiguousarray(Wp.T).astype(dt_np)
            m["bias"] = bp.reshape(1, D)
        else:
            m["wpT"] = np.ascontiguousarray(Wp[:, rows].T).astype(dt_np)
            m["bias"] = (bp if g == 0 else np.zeros_like(bp)).reshape(1, D)
        in_maps.append(m)
    return in_maps


def kernel(x, Wq, Wk, Wv, Wp, bp, _trace=False, _mode=None):
    mode = _mode or MODE
    nc = _get_nc(mode)
    in_maps = _make_in_maps(x, Wq, Wk, Wv, Wp, bp, mode)
    res = _run_spmd(nc, in_maps, trace=_trace)
    out = np.empty((B, T, D), dtype=np.float32)
    for b in range(B):
        if mode == "ag":
            out[b, 0:QH] = res.results[2 * b]["y"][0:QH]
            out[b, QH:T] = res.results[2 * b + 1]["y"][QH:T]
        else:
            out[b] = res.results[2 * b]["y"] + res.results[2 * b + 1]["y"]
    if _trace:
        kernel.last_results = res
    return out



# revision 8
# speedup vs baseline: 1.0439x; 1.0439x over previous
"""Trainium2 Bass kernel for causal multi-head attention + output projection.

Problem (hardcoded): x[4, 2048, 1024] fp32, 16 heads, head_dim 64, causal,
torch-Linear convention (y = x @ W.T), output projection with bias.

Sharding over 8 NeuronCores: batch (4) x head-group (2 groups of 8 heads).
Each core computes q/k/v for its 8 heads of its batch, causal attention in
the S^T layout (keys on partitions, queries on free dim; softmax denominators
produced by an appended ones-column in V), then a PARTIAL output projection
over its own 8 heads (contraction 512) for all 2048 queries. A chunked
pairwise ReduceScatter sums the partials and leaves each core with its own
1024-query half of the final output; the host concatenates rows.

Attention is a single-head software pipeline: per step t=(head, qh, j) the
PE computes S(t), ACT exponentiates into SBUF, and the PE applies AV(t-1)
(lagged one step so the in-order PE queue never blocks the ACT engine, which
is the bottleneck at ~1 elem/lane/cycle over the whole causal area).

All matmuls run as float32r (TF32-like, ~1.5e-4 rel err).
"""
import os
import sys
import types

import numpy as np

import concourse.bass as bass
import concourse.mybir as mybir
import concourse.tile as tile
from concourse import bacc, bass_utils

DT = getattr(mybir.dt, os.environ.get("ATTN_DT_MAIN", "float32r"))
F32 = mybir.dt.float32
AF = mybir.ActivationFunctionType
OP = mybir.AluOpType

B, T, D = 4, 2048, 1024
H, HD = 16, 64
HG = 8          # heads per core
QH = T // 2     # query half
N_CORES = 8
SCALE = 1.0 / 8.0

ADT_NAME = os.environ.get("ATTN_DTYPE", "float32r")
ADT = getattr(mybir.dt, ADT_NAME)

RG_PAIRS = [[0, 1], [2, 3], [4, 5], [6, 7]]


# ---------------------------------------------------------------------------
# environment glue
# ---------------------------------------------------------------------------

def _install_ntff_hook():
    if 'antenv.axon_hooks' in sys.modules:
        return
    try:
        from trn_agent_boot.trn_boot import _ntff_profile_via_ctypes
        hook = _ntff_profile_via_ctypes('/opt/axon/libaxon_pjrt.so')
    except Exception:
        hook = None
    mod = types.ModuleType('antenv.axon_hooks')
    mod.get_axon_ntff_profile_hook = lambda: hook
    mod.set_axon_ntff_profile_hook = lambda h: None
    sys.modules['antenv.axon_hooks'] = mod


def _run_spmd(nc, in_maps, trace=False):
    from concourse.bass_interp import get_hw_module
    bass_utils.upload_artifacts = lambda tmpdir: tmpdir
    if trace:
        _install_ntff_hook()
    old_m = nc.m
    nc.m = get_hw_module(nc.m)
    try:
        return bass_utils.run_bass_kernel_spmd(
            nc, in_maps, core_ids=list(range(N_CORES)),
            trace=trace, trace_cores=[0] if trace else None,
        )
    finally:
        nc.m = old_m


# ---------------------------------------------------------------------------
# kernel program
# ---------------------------------------------------------------------------

def _qkv_phase(nc, tc, ctx, xT, wqT, wkT, wvT, vone, qT_sb, kT_sb, v_sb):
    """Compute q.T [512,2048], k.T [512,2048] and V' [2048, 8, 65] for this
    core's 8 heads. Contraction dim D lives on partitions; all operands fp32r."""
    xp = ctx.enter_context(tc.tile_pool(name="xph", bufs=16))
    wp = ctx.enter_context(tc.tile_pool(name="wph", bufs=16))
    ps = ctx.enter_context(tc.tile_pool(name="p2ps", bufs=2, space="PSUM"))

    xT_r = xT.rearrange("(ko ki) t -> ki ko t", ki=128)

    def load_w(wT):
        parts = []
        wT_r = wT.rearrange("(ko ki) n -> ki ko n", ki=128)
        for kk in range(8):
            t = wp.tile([128, 512], DT, tag="w")
            nc.sync.dma_start(t[:], wT_r[:, kk])
            parts.append(t)
        return parts

    # k.T first: emit its weight slices and the x slices in consumption order
    # so the first matmuls start as soon as ~1MB has landed.
    wk_sb = load_w(wkT)
    xh = [[None] * 8 for _ in range(2)]
    for kk in range(8):
        for half in range(2):
            t = xp.tile([128, QH], DT, tag="xh")
            nc.sync.dma_start(t[:], xT_r[:, kk, half * QH:(half + 1) * QH])
            xh[half][kk] = t
    for m in range(4):
        for half in range(2):
            pt = ps.tile([128, QH], F32, tag="st")
            for nch in range(2):
                sl = slice(nch * 512, (nch + 1) * 512)
                for kk in range(8):
                    nc.tensor.matmul(
                        pt[:, sl],
                        lhsT=wk_sb[kk][:, m * 128:(m + 1) * 128],
                        rhs=xh[half][kk][:, sl],
                        start=(kk == 0), stop=(kk == 7))
            nc.vector.tensor_copy(kT_sb[:, m, half * QH:(half + 1) * QH], pt[:])

    # q for head pair 0 early so attention can begin before V finishes.
    wq_sb = load_w(wqT)
    for half in range(2):
        pt = ps.tile([128, QH], F32, tag="st")
        for nch in range(2):
            sl = slice(nch * 512, (nch + 1) * 512)
            for kk in range(8):
                nc.tensor.matmul(
                    pt[:, sl],
                    lhsT=wq_sb[kk][:, 0:128],
                    rhs=xh[half][kk][:, sl],
                    start=(kk == 0), stop=(kk == 7))
        nc.vector.tensor_copy(qT_sb[:, 0, half * QH:(half + 1) * QH], pt[:])

    wv_sb = load_w(wvT)
    nc.sync.dma_start(v_sb[:, :, :, 64],
                      vone.rearrange("p (a b) -> p a b", a=16))
    for m in range(16):
        pt = ps.tile([128, QH], F32, tag="st")
        for kk in range(8):
            nc.tensor.matmul(
                pt[:, 0:512],
                lhsT=xh[m // 8][kk][:, (m % 8) * 128:(m % 8 + 1) * 128],
                rhs=wv_sb[kk][:],
                start=(kk == 0), stop=(kk == 7))
        nc.vector.tensor_copy(
            v_sb[:, m, :, 0:64],
            pt[:, 0:512].rearrange("p (h d) -> p h d", h=HG))

    for m in range(1, 4):
        for half in range(2):
            pt = ps.tile([128, QH], F32, tag="st")
            for nch in range(2):
                sl = slice(nch * 512, (nch + 1) * 512)
                for kk in range(8):
                    nc.tensor.matmul(
                        pt[:, sl],
                        lhsT=wq_sb[kk][:, m * 128:(m + 1) * 128],
                        rhs=xh[half][kk][:, sl],
                        start=(kk == 0), stop=(kk == 7))
            nc.vector.tensor_copy(qT_sb[:, m, half * QH:(half + 1) * QH], pt[:])


def build_nc():
    nc = bacc.Bacc("TRN2", target_bir_lowering=False, debug=False,
                   enable_asserts=False, num_devices=N_CORES)
    xT = nc.dram_tensor("xT", [D, T], DT, kind="ExternalInput").ap()
    wqT = nc.dram_tensor("wqT", [D, 512], DT, kind="ExternalInput").ap()
    wkT = nc.dram_tensor("wkT", [D, 512], DT, kind="ExternalInput").ap()
    wvT = nc.dram_tensor("wvT", [D, 512], DT, kind="ExternalInput").ap()
    wpT = nc.dram_tensor("wpT", [512, D], DT, kind="ExternalInput").ap()
    mask = nc.dram_tensor("mask", [128, 128], ADT, kind="ExternalInput").ap()
    vone = nc.dram_tensor("vone", [128, 128], ADT, kind="ExternalInput").ap()
    bias = nc.dram_tensor("bias", [1, D], F32, kind="ExternalInput").ap()
    snum = nc.dram_tensor("snum", [16, QH], F32).ap()
    srec = nc.dram_tensor("srec", [16, QH], F32).ap()
    rsin = [nc.dram_tensor(f"rsin{i}", [2, 128, D], F32).ap() for i in range(8)]
    yint = nc.dram_tensor("yint", [8, 128, D], F32).ap()
    yo = nc.dram_tensor("yo", [8, 128, D], F32, kind="ExternalOutput").ap()

    from contextlib import ExitStack
    with tile.TileContext(nc) as tc, ExitStack() as ctx:
        per = ctx.enter_context(tc.tile_pool(name="per", bufs=1))

        qT_sb = per.tile([128, 4, T], ADT, tag="qT")
        kT_sb = per.tile([128, 4, T], ADT, tag="kT")
        v_sb = per.tile([128, 16, HG, 65], ADT, tag="v")
        mask_sb = per.tile([128, 128], ADT, tag="mask")

        nc.sync.dma_start(mask_sb[:], mask[:])

        with ExitStack() as p2:
            _qkv_phase(nc, tc, p2, xT, wqT, wkT, wvT, vone, qT_sb, kT_sb, v_sb)

        mid = ctx.enter_context(tc.tile_pool(name="mid", bufs=1))
        o_all = mid.tile([128, 4, T], ADT, tag="oacc")
        wp_sb = mid.tile([128, 4, D], DT, tag="wp")
        nc.sync.dma_start(wp_sb[:],
                          wpT.rearrange("(ko ki) n -> ki ko n", ki=128))
        bias_bc = mid.tile([128, D], F32, tag="bbc")
        nc.sync.dma_start(bias_bc[:], bias[0][None, :].broadcast_to([128, D]))

        # ---------------- attention: flattened single-head pipeline --------
        # steps: (h, qh, j); S(t)+exp(t) emitted at step t, AV(t-1) lagged.
        steps = []
        for h in range(HG):
            for qh in range(2):
                for j in range(8 * qh + 8):
                    steps.append((h, qh, j))
        n_steps = len(steps)

        with ExitStack() as attn:
            sps = attn.enter_context(tc.tile_pool(name="sps", bufs=2, space="PSUM"))
            ops = attn.enter_context(tc.tile_pool(name="ops", bufs=2, space="PSUM"))
            es = attn.enter_context(tc.tile_pool(name="es", bufs=3))
            ev = attn.enter_context(tc.tile_pool(name="ev", bufs=2))
            nrm = attn.enter_context(tc.tile_pool(name="nrm", bufs=2))

            e_tiles = [None] * n_steps   # (e_sb, n, coff)
            o_tiles = {}                 # (h, qh) -> psum tile

            def step_params(h, qh, j):
                qstart = max(QH * qh, 128 * j)
                n = QH * (qh + 1) - qstart
                coff = qstart - QH * qh
                return qstart, n, coff

            def emit_S_exp(t):
                h, qh, j = steps[t]
                qstart, n, coff = step_params(h, qh, j)
                pbase = 64 * (h % 2)
                sub = h // 2
                s_ps = sps.tile([128, QH], F32, tag="s", name=f"s{t}")
                for c in range(0, n, 512):
                    cn = min(512, n - c)
                    nc.tensor.matmul(
                        s_ps[:, c:c + cn],
                        lhsT=kT_sb[pbase:pbase + 64, sub, j * 128:(j + 1) * 128],
                        rhs=qT_sb[pbase:pbase + 64, sub,
                                  qstart + c:qstart + c + cn],
                        start=True, stop=True)
                e_sb = es.tile([128, QH], ADT, tag="e", name=f"e{t}")
                nc.scalar.activation(e_sb[:, 0:n], s_ps[:, 0:n], AF.Exp,
                                     scale=SCALE)
                if j >= 8 * qh:
                    nc.vector.tensor_tensor(
                        e_sb[:, 0:128], e_sb[:, 0:128], mask_sb[:], OP.mult)
                e_tiles[t] = (e_sb, n, coff)

            def emit_AV(t):
                h, qh, j = steps[t]
                e_sb, n, coff = e_tiles[t]
                if j == 0:
                    o_tiles[(h, qh)] = ops.tile([65, QH], F32, tag="o",
                                                name=f"o{h}_{qh}")
                o_ps = o_tiles[(h, qh)]
                jmax = 8 * qh + 8
                c0 = coff
                while c0 < QH:
                    hi = min(QH, (c0 // 512 + 1) * 512)
                    nc.tensor.matmul(
                        o_ps[:, c0:hi],
                        lhsT=v_sb[:, j, h, :],
                        rhs=e_sb[:, c0 - coff:hi - coff],
                        start=(j == 0), stop=(j == jmax - 1),
                        skip_group_check=True)
                    c0 = hi
                e_tiles[t] = None

            def emit_evict(h, qh):
                # o rows 0..63 -> o_all (odd heads go via an SBUF temp since
                # only DMA can shift partitions), row 64 -> snum.
                o_ps = o_tiles.pop((h, qh))
                pbase = 64 * (h % 2)
                sub = h // 2
                i = 2 * h + qh
                dtile = ev.tile([1, QH], F32, tag="dn", name=f"dn{h}_{qh}")
                nc.vector.tensor_copy(dtile[:], o_ps[64:65, :])
                nc.sync.dma_start(snum[i:i + 1, :], dtile[:])
                if pbase == 0:
                    nc.vector.tensor_copy(
                        o_all[0:64, sub, QH * qh:QH * (qh + 1)], o_ps[0:64, :])
                else:
                    tmp = ev.tile([64, QH], ADT, tag="ev", name=f"ev{h}_{qh}")
                    nc.vector.tensor_copy(tmp[:], o_ps[0:64, :])
                    nc.sync.dma_start(
                        o_all[64:128, sub, QH * qh:QH * (qh + 1)], tmp[:])
                # reciprocal of the denominators, spread over 64 partitions
                st64 = nrm.tile([64, QH // 64], F32, tag="sp")
                nc.sync.dma_start(st64[:], snum[i].rearrange("(p f) -> p f", p=64))
                nc.vector.reciprocal(st64[:], st64[:])
                nc.sync.dma_start(srec[i].rearrange("(p f) -> p f", p=64), st64[:])
                bc = nrm.tile([128, QH], F32, tag="bc")
                nc.sync.dma_start(bc[:], srec[i][None, :].broadcast_to([128, QH]))
                sl_ap = o_all[pbase:pbase + 64, sub, QH * qh:QH * (qh + 1)]
                nc.vector.tensor_tensor(sl_ap, sl_ap, bc[pbase:pbase + 64, :],
                                        OP.mult)

            for t in range(n_steps):
                emit_S_exp(t)
                if t > 0:
                    emit_AV(t - 1)
                    h0, qh0, j0 = steps[t - 1]
                    if j0 == 8 * qh0 + 7:
                        emit_evict(h0, qh0)
            emit_AV(n_steps - 1)
            emit_evict(*steps[n_steps - 1][:2])

        # ---------------- partial projection + chunked ReduceScatter -------
        yop = ctx.enter_context(tc.tile_pool(name="yop", bufs=3))
        pps = ctx.enter_context(tc.tile_pool(name="pps", bufs=2, space="PSUM"))
        for i in range(8):
            for g in range(2):
                m = 8 * g + i
                yp = pps.tile([128, D], F32, tag="yp")
                for nch in range(2):
                    sl = slice(nch * 512, (nch + 1) * 512)
                    for kk in range(4):
                        nc.tensor.matmul(
                            yp[:, sl],
                            lhsT=o_all[:, kk, m * 128:(m + 1) * 128],
                            rhs=wp_sb[:, kk, sl],
                            start=(kk == 0), stop=(kk == 3))
                y_sb = yop.tile([128, D], F32, tag="y")
                nc.vector.tensor_tensor(y_sb[:], yp[:], bias_bc[:], OP.add)
                nc.sync.dma_start(rsin[i][g], y_sb[:])
            nc.gpsimd.collective_compute(
                "ReduceScatter", OP.add,
                replica_groups=RG_PAIRS,
                ins=[rsin[i][:]], outs=[yint[i]],
            )
            nc.sync.dma_start(yo[i], yint[i])

    nc.compile()
    return nc


# ---------------------------------------------------------------------------
# host-side sharding + entry point
# ---------------------------------------------------------------------------

_NC_CACHE = {}


def _get_nc():
    if "rs" not in _NC_CACHE:
        _NC_CACHE["rs"] = build_nc()
    return _NC_CACHE["rs"]


def _make_in_maps(x, Wq, Wk, Wv, Wp, bp):
    x = np.asarray(x, dtype=np.float32)
    Wq = np.asarray(Wq, dtype=np.float32)
    Wk = np.asarray(Wk, dtype=np.float32)
    Wv = np.asarray(Wv, dtype=np.float32)
    Wp = np.asarray(Wp, dtype=np.float32)
    bp = np.asarray(bp, dtype=np.float32)

    adt_np = mybir.dt.np(ADT)
    dt_np = mybir.dt.np(DT)
    mask = np.zeros((128, 128), dtype=np.float32)
    k_idx = np.arange(128)[:, None]
    q_idx = np.arange(128)[None, :]
    mask[q_idx >= k_idx] = 1.0
    mask = mask.astype(adt_np)

    xTs = [np.ascontiguousarray(x[b].T) for b in range(B)]
    in_maps = []
    for c in range(N_CORES):
        b, g = c // 2, c % 2
        rows = slice(512 * g, 512 * (g + 1))
        m = {
            "xT": xTs[b].astype(dt_np),
            "wqT": np.ascontiguousarray(Wq[rows, :].T).astype(dt_np),
            "wkT": np.ascontiguousarray(Wk[rows, :].T).astype(dt_np),
            "wvT": np.ascontiguousarray(Wv[rows, :].T).astype(dt_np),
            "wpT": np.ascontiguousarray(Wp[:, rows].T).astype(dt_np),
            "mask": mask,
            "vone": np.ones((128, 128), dtype=adt_np),
            "bias": (bp if g == 0 else np.zeros_like(bp)).reshape(1, D),
        }
        in_maps.append(m)
    return in_maps


def kernel(x, Wq, Wk, Wv, Wp, bp, _trace=False):
    nc = _get_nc()
    in_maps = _make_in_maps(x, Wq, Wk, Wv, Wp, bp)
    res = _run_spmd(nc, in_maps, trace=_trace)
    out = np.empty((B, T, D), dtype=np.float32)
    for b in range(B):
        out[b, 0:QH] = res.results[2 * b]["yo"].reshape(QH, D)
        out[b, QH:T] = res.results[2 * b + 1]["yo"].reshape(QH, D)
    if _trace:
        kernel.last_results = res
    return out


# revision 9
# speedup vs baseline: 1.9273x; 1.8463x over previous
"""Trainium2 Bass kernel for causal multi-head attention + output projection.

Problem (hardcoded): x[4, 2048, 1024] fp32, 16 heads, head_dim 64, causal,
torch-Linear convention (y = x @ W.T), output projection with bias.

Sharding over 8 NeuronCores: batch (4) x head-group (2 groups of 8 heads).
Each core computes q/k/v for its 8 heads of its batch, causal attention in
the S^T layout (keys on partitions, queries on free dim; softmax denominators
produced by an appended ones-column in V), then a PARTIAL output projection
over its own 8 heads (contraction 512) for all 2048 queries. The host sums
the two partial projections of each batch (the tensor-parallel all-reduce,
done host-side because on-device collectives on this fabric run ~30 GB/s);
optionally a chunked pairwise ReduceScatter does it on device
(ATTN_COMBINE=rs).

Attention runs a single-head software pipeline: per step t=(head, qh, j) the
PE computes S(t), ACT exponentiates into SBUF, and the PE applies AV(t-1)
(lagged one step so the in-order PE queue stays ahead of ACT). kT is stored
zero-padded per head on the full 128 partitions so S and AV share one PE
tile mode (128-contraction) — mode switches would expose every LDWEIGHTS.

QKV + projection matmuls run fp32r; attention operands (q/k/v/e) are bf16.
"""
import os
import sys
import types

import numpy as np

import concourse.bass as bass
import concourse.mybir as mybir
import concourse.tile as tile
from concourse import bacc, bass_utils

DT = getattr(mybir.dt, os.environ.get("ATTN_DT_MAIN", "float32r"))
ODT = mybir.dt.float32r     # o_all accumulator dtype (proj lhsT)
F32 = mybir.dt.float32
AF = mybir.ActivationFunctionType
OP = mybir.AluOpType

B, T, D = 4, 2048, 1024
H, HD = 16, 64
HG = 8          # heads per core
QH = T // 2     # query half
N_CORES = 8
SCALE = 1.0 / 8.0

ADT_NAME = os.environ.get("ATTN_DTYPE", "bfloat16")
ADT = getattr(mybir.dt, ADT_NAME)
COMBINE = os.environ.get("ATTN_COMBINE", "hostsum")

RG_PAIRS = [[0, 1], [2, 3], [4, 5], [6, 7]]


# ---------------------------------------------------------------------------
# environment glue
# ---------------------------------------------------------------------------

def _install_ntff_hook():
    if 'antenv.axon_hooks' in sys.modules:
        return
    try:
        from trn_agent_boot.trn_boot import _ntff_profile_via_ctypes
        hook = _ntff_profile_via_ctypes('/opt/axon/libaxon_pjrt.so')
    except Exception:
        hook = None
    mod = types.ModuleType('antenv.axon_hooks')
    mod.get_axon_ntff_profile_hook = lambda: hook
    mod.set_axon_ntff_profile_hook = lambda h: None
    sys.modules['antenv.axon_hooks'] = mod


def _run_spmd(nc, in_maps, trace=False):
    from concourse.bass_interp import get_hw_module
    bass_utils.upload_artifacts = lambda tmpdir: tmpdir
    if trace:
        _install_ntff_hook()
    old_m = nc.m
    nc.m = get_hw_module(nc.m)
    try:
        return bass_utils.run_bass_kernel_spmd(
            nc, in_maps, core_ids=list(range(N_CORES)),
            trace=trace, trace_cores=[0] if trace else None,
        )
    finally:
        nc.m = old_m


# ---------------------------------------------------------------------------
# kernel program
# ---------------------------------------------------------------------------

def _qkv_phase(nc, tc, ctx, xT, wqT, wkT, wvT, vone, qT_sb, kT_sb, v_sb):
    """Compute q.T (packed, 2 heads per 128 partitions), zero-padded k.T
    (one head per 128 partitions, dims on rows 64h..64h+64 matching q's
    slot) and V' [2048, 8, 65] for this core's 8 heads."""
    xp = ctx.enter_context(tc.tile_pool(name="xph", bufs=16))
    wp = ctx.enter_context(tc.tile_pool(name="wph", bufs=16))
    ps = ctx.enter_context(tc.tile_pool(name="p2ps", bufs=2, space="PSUM"))

    xT_r = xT.rearrange("(ko ki) t -> ki ko t", ki=128)

    def load_w(wT):
        parts = []
        wT_r = wT.rearrange("(ko ki) n -> ki ko n", ki=128)
        for kk in range(8):
            t = wp.tile([128, 512], DT, tag="w")
            nc.sync.dma_start(t[:], wT_r[:, kk])
            parts.append(t)
        return parts

    # k.T first: emit its weight slices and the x slices in consumption order
    # so the first matmuls start as soon as ~1MB has landed.
    wk_sb = load_w(wkT)
    xh = [[None] * 8 for _ in range(2)]
    for kk in range(8):
        for half in range(2):
            t = xp.tile([128, QH], DT, tag="xh")
            nc.sync.dma_start(t[:], xT_r[:, kk, half * QH:(half + 1) * QH])
            xh[half][kk] = t
    for m in range(4):
        for half in range(2):
            pt = ps.tile([128, QH], F32, tag="st")
            for nch in range(2):
                sl = slice(nch * 512, (nch + 1) * 512)
                for kk in range(8):
                    nc.tensor.matmul(
                        pt[:, sl],
                        lhsT=wk_sb[kk][:, m * 128:(m + 1) * 128],
                        rhs=xh[half][kk][:, sl],
                        start=(kk == 0), stop=(kk == 7))
            tsl = slice(half * QH, (half + 1) * QH)
            nc.vector.tensor_copy(kT_sb[0:64, 2 * m, tsl], pt[0:64, :])
            nc.vector.tensor_copy(kT_sb[64:128, 2 * m + 1, tsl], pt[64:128, :])

    # q for head pair 0 early so attention can begin before V finishes.
    wq_sb = load_w(wqT)
    for half in range(2):
        pt = ps.tile([128, QH], F32, tag="st")
        for nch in range(2):
            sl = slice(nch * 512, (nch + 1) * 512)
            for kk in range(8):
                nc.tensor.matmul(
                    pt[:, sl],
                    lhsT=wq_sb[kk][:, 0:128],
                    rhs=xh[half][kk][:, sl],
                    start=(kk == 0), stop=(kk == 7))
        nc.vector.tensor_copy(qT_sb[:, 0, half * QH:(half + 1) * QH], pt[:])

    wv_sb = load_w(wvT)
    nc.sync.dma_start(v_sb[:, :, :, 64],
                      vone.rearrange("p (a b) -> p a b", a=16))
    for m in range(16):
        pt = ps.tile([128, QH], F32, tag="st")
        for kk in range(8):
            nc.tensor.matmul(
                pt[:, 0:512],
                lhsT=xh[m // 8][kk][:, (m % 8) * 128:(m % 8 + 1) * 128],
                rhs=wv_sb[kk][:],
                start=(kk == 0), stop=(kk == 7))
        nc.vector.tensor_copy(
            v_sb[:, m, :, 0:64],
            pt[:, 0:512].rearrange("p (h d) -> p h d", h=HG))

    for m in range(1, 4):
        for half in range(2):
            pt = ps.tile([128, QH], F32, tag="st")
            for nch in range(2):
                sl = slice(nch * 512, (nch + 1) * 512)
                for kk in range(8):
                    nc.tensor.matmul(
                        pt[:, sl],
                        lhsT=wq_sb[kk][:, m * 128:(m + 1) * 128],
                        rhs=xh[half][kk][:, sl],
                        start=(kk == 0), stop=(kk == 7))
            nc.vector.tensor_copy(qT_sb[:, m, half * QH:(half + 1) * QH], pt[:])


def build_nc(combine):
    nc = bacc.Bacc("TRN2", target_bir_lowering=False, debug=False,
                   enable_asserts=False, num_devices=N_CORES)
    xT = nc.dram_tensor("xT", [D, T], DT, kind="ExternalInput").ap()
    wqT = nc.dram_tensor("wqT", [D, 512], DT, kind="ExternalInput").ap()
    wkT = nc.dram_tensor("wkT", [D, 512], DT, kind="ExternalInput").ap()
    wvT = nc.dram_tensor("wvT", [D, 512], DT, kind="ExternalInput").ap()
    wpT = nc.dram_tensor("wpT", [512, D], DT, kind="ExternalInput").ap()
    mask = nc.dram_tensor("mask", [128, 128], ADT, kind="ExternalInput").ap()
    vone = nc.dram_tensor("vone", [128, 128], ADT, kind="ExternalInput").ap()
    bias = nc.dram_tensor("bias", [1, D], F32, kind="ExternalInput").ap()
    snum = nc.dram_tensor("snum", [16, QH], F32).ap()
    srec = nc.dram_tensor("srec", [16, QH], F32).ap()
    if combine == "rs":
        rsin = [nc.dram_tensor(f"rsin{i}", [2, 128, D], F32).ap()
                for i in range(8)]
        yint = nc.dram_tensor("yint", [8, 128, D], F32).ap()
        yo = nc.dram_tensor("yo", [8, 128, D], F32, kind="ExternalOutput").ap()
    else:
        yo = nc.dram_tensor("yo", [T, D], F32, kind="ExternalOutput").ap()

    from contextlib import ExitStack
    with tile.TileContext(nc) as tc, ExitStack() as ctx:
        per = ctx.enter_context(tc.tile_pool(name="per", bufs=1))

        qT_sb = per.tile([128, 4, T], ADT, tag="qT")
        kT_sb = per.tile([128, 8, T], ADT, tag="kT")
        v_sb = per.tile([128, 16, HG, 65], ADT, tag="v")
        mask_sb = per.tile([128, 128], ADT, tag="mask")

        nc.sync.dma_start(mask_sb[:], mask[:])
        # zero the dead half of each head's kT slot (gpsimd is otherwise idle)
        nc.gpsimd.memset(kT_sb[:], 0.0)

        with ExitStack() as p2:
            _qkv_phase(nc, tc, p2, xT, wqT, wkT, wvT, vone, qT_sb, kT_sb, v_sb)

        mid = ctx.enter_context(tc.tile_pool(name="mid", bufs=1))
        o_all = mid.tile([128, 4, T], ODT, tag="oacc")
        wp_sb = mid.tile([128, 4, D], DT, tag="wp")
        nc.sync.dma_start(wp_sb[:],
                          wpT.rearrange("(ko ki) n -> ki ko n", ki=128))
        bias_bc = mid.tile([128, D], F32, tag="bbc")
        nc.sync.dma_start(bias_bc[:], bias[0][None, :].broadcast_to([128, D]))

        # ---------------- attention: flattened single-head pipeline --------
        # steps: (h, qh, j); S(t)+exp(t) emitted at step t, AV(t-1) lagged.
        steps = []
        for h in range(HG):
            for qh in range(2):
                for j in range(8 * qh + 8):
                    steps.append((h, qh, j))
        n_steps = len(steps)

        with ExitStack() as attn:
            sps = attn.enter_context(tc.tile_pool(name="sps", bufs=2, space="PSUM"))
            ops = attn.enter_context(tc.tile_pool(name="ops", bufs=2, space="PSUM"))
            es = attn.enter_context(tc.tile_pool(name="es", bufs=4))
            ev = attn.enter_context(tc.tile_pool(name="ev", bufs=2))
            nrm = attn.enter_context(tc.tile_pool(name="nrm", bufs=2))

            e_tiles = [None] * n_steps   # (e_sb, n, coff)
            o_tiles = {}                 # (h, qh) -> psum tile

            def step_params(h, qh, j):
                qstart = max(QH * qh, 128 * j)
                n = QH * (qh + 1) - qstart
                coff = qstart - QH * qh
                return qstart, n, coff

            def emit_S_exp(t):
                h, qh, j = steps[t]
                qstart, n, coff = step_params(h, qh, j)
                sub = h // 2
                s_ps = sps.tile([128, QH], F32, tag="s", name=f"s{t}")
                for c in range(0, n, 512):
                    cn = min(512, n - c)
                    nc.tensor.matmul(
                        s_ps[:, c:c + cn],
                        lhsT=kT_sb[:, h, j * 128:(j + 1) * 128],
                        rhs=qT_sb[:, sub, qstart + c:qstart + c + cn],
                        start=True, stop=True)
                e_sb = es.tile([128, QH], ADT, tag="e", name=f"e{t}")
                nc.scalar.activation(e_sb[:, 0:n], s_ps[:, 0:n], AF.Exp,
                                     scale=SCALE)
                if j >= 8 * qh:
                    nc.vector.tensor_tensor(
                        e_sb[:, 0:128], e_sb[:, 0:128], mask_sb[:], OP.mult)
                e_tiles[t] = (e_sb, n, coff)

            def emit_AV(t):
                h, qh, j = steps[t]
                e_sb, n, coff = e_tiles[t]
                if j == 0:
                    o_tiles[(h, qh)] = ops.tile([65, QH], F32, tag="o",
                                                name=f"o{h}_{qh}")
                o_ps = o_tiles[(h, qh)]
                jmax = 8 * qh + 8
                c0 = coff
                while c0 < QH:
                    hi = min(QH, (c0 // 512 + 1) * 512)
                    nc.tensor.matmul(
                        o_ps[:, c0:hi],
                        lhsT=v_sb[:, j, h, :],
                        rhs=e_sb[:, c0 - coff:hi - coff],
                        start=(j == 0), stop=(j == jmax - 1),
                        skip_group_check=True)
                    c0 = hi
                e_tiles[t] = None

            def emit_evict(h, qh):
                # o rows 0..63 -> o_all (odd heads go via an SBUF temp since
                # only DMA can shift partitions), row 64 -> snum.
                o_ps = o_tiles.pop((h, qh))
                pbase = 64 * (h % 2)
                sub = h // 2
                i = 2 * h + qh
                dtile = ev.tile([1, QH], F32, tag="dn", name=f"dn{h}_{qh}")
                nc.vector.tensor_copy(dtile[:], o_ps[64:65, :])
                nc.sync.dma_start(snum[i:i + 1, :], dtile[:])
                if pbase == 0:
                    nc.vector.tensor_copy(
                        o_all[0:64, sub, QH * qh:QH * (qh + 1)], o_ps[0:64, :])
                else:
                    tmp = ev.tile([64, QH], ODT, tag="ev", name=f"ev{h}_{qh}")
                    nc.vector.tensor_copy(tmp[:], o_ps[0:64, :])
                    nc.sync.dma_start(
                        o_all[64:128, sub, QH * qh:QH * (qh + 1)], tmp[:])
                # reciprocal of the denominators, spread over 64 partitions
                st64 = nrm.tile([64, QH // 64], F32, tag="sp")
                nc.sync.dma_start(st64[:], snum[i].rearrange("(p f) -> p f", p=64))
                nc.vector.reciprocal(st64[:], st64[:])
                nc.sync.dma_start(srec[i].rearrange("(p f) -> p f", p=64), st64[:])
                bc = nrm.tile([128, QH], F32, tag="bc")
                nc.sync.dma_start(bc[:], srec[i][None, :].broadcast_to([128, QH]))
                sl_ap = o_all[pbase:pbase + 64, sub, QH * qh:QH * (qh + 1)]
                nc.vector.tensor_tensor(sl_ap, sl_ap, bc[pbase:pbase + 64, :],
                                        OP.mult)

            for t in range(n_steps):
                emit_S_exp(t)
                if t > 0:
                    emit_AV(t - 1)
                    h0, qh0, j0 = steps[t - 1]
                    if j0 == 8 * qh0 + 7:
                        emit_evict(h0, qh0)
            emit_AV(n_steps - 1)
            emit_evict(*steps[n_steps - 1][:2])

        # ---------------- partial projection -------------------------------
        yop = ctx.enter_context(tc.tile_pool(name="yop", bufs=3))
        pps = ctx.enter_context(tc.tile_pool(name="pps", bufs=2, space="PSUM"))
        for i in range(8):
            for g in range(2):
                m = 8 * g + i
                yp = pps.tile([128, D], F32, tag="yp")
                for nch in range(2):
                    sl = slice(nch * 512, (nch + 1) * 512)
                    for kk in range(4):
                        nc.tensor.matmul(
                            yp[:, sl],
                            lhsT=o_all[:, kk, m * 128:(m + 1) * 128],
                            rhs=wp_sb[:, kk, sl],
                            start=(kk == 0), stop=(kk == 3))
                y_sb = yop.tile([128, D], F32, tag="y")
                nc.vector.tensor_tensor(y_sb[:], yp[:], bias_bc[:], OP.add)
                if combine == "rs":
                    nc.sync.dma_start(rsin[i][g], y_sb[:])
                else:
                    nc.sync.dma_start(yo[m * 128:(m + 1) * 128, :], y_sb[:])
            if combine == "rs":
                nc.gpsimd.collective_compute(
                    "ReduceScatter", OP.add,
                    replica_groups=RG_PAIRS,
                    ins=[rsin[i][:]], outs=[yint[i]],
                )
                nc.sync.dma_start(yo[i], yint[i])

    nc.compile()
    return nc


# ---------------------------------------------------------------------------
# host-side sharding + entry point
# ---------------------------------------------------------------------------

_NC_CACHE = {}


def _get_nc(combine):
    if combine not in _NC_CACHE:
        _NC_CACHE[combine] = build_nc(combine)
    return _NC_CACHE[combine]


def _make_in_maps(x, Wq, Wk, Wv, Wp, bp):
    x = np.asarray(x, dtype=np.float32)
    Wq = np.asarray(Wq, dtype=np.float32)
    Wk = np.asarray(Wk, dtype=np.float32)
    Wv = np.asarray(Wv, dtype=np.float32)
    Wp = np.asarray(Wp, dtype=np.float32)
    bp = np.asarray(bp, dtype=np.float32)

    adt_np = mybir.dt.np(ADT)
    dt_np = mybir.dt.np(DT)
    mask = np.zeros((128, 128), dtype=np.float32)
    k_idx = np.arange(128)[:, None]
    q_idx = np.arange(128)[None, :]
    mask[q_idx >= k_idx] = 1.0
    mask = mask.astype(adt_np)

    xTs = [np.ascontiguousarray(x[b].T) for b in range(B)]
    in_maps = []
    for c in range(N_CORES):
        b, g = c // 2, c % 2
        rows = slice(512 * g, 512 * (g + 1))
        m = {
            "xT": xTs[b].astype(dt_np),
            "wqT": np.ascontiguousarray(Wq[rows, :].T).astype(dt_np),
            "wkT": np.ascontiguousarray(Wk[rows, :].T).astype(dt_np),
            "wvT": np.ascontiguousarray(Wv[rows, :].T).astype(dt_np),
            "wpT": np.ascontiguousarray(Wp[:, rows].T).astype(dt_np),
            "mask": mask,
            "vone": np.ones((128, 128), dtype=adt_np),
            "bias": (bp if g == 0 else np.zeros_like(bp)).reshape(1, D),
        }
        in_maps.append(m)
    return in_maps


def kernel(x, Wq, Wk, Wv, Wp, bp, _trace=False):
    combine = COMBINE
    nc = _get_nc(combine)
    in_maps = _make_in_maps(x, Wq, Wk, Wv, Wp, bp)
    res = _run_spmd(nc, in_maps, trace=_trace)
    out = np.empty((B, T, D), dtype=np.float32)
    for b in range(B):
        ya = res.results[2 * b]["yo"]
        yb = res.results[2 * b + 1]["yo"]
        if combine == "rs":
            out[b, 0:QH] = ya.reshape(QH, D)
            out[b, QH:T] = yb.reshape(QH, D)
        else:
            out[b] = ya + yb
    if _trace:
        kernel.last_results = res
    return out


# revision 13
# speedup vs baseline: 1.9571x; 1.0155x over previous
"""Trainium2 Bass kernel for causal multi-head attention + output projection.

Problem (hardcoded): x[4, 2048, 1024] fp32, 16 heads, head_dim 64, causal,
torch-Linear convention (y = x @ W.T), output projection with bias.

Sharding over 8 NeuronCores: batch (4) x head-group (2 groups of 8 heads).
Each core computes q/k/v for its 8 heads of its batch, causal attention in
the S^T layout (keys on partitions, queries on free dim; softmax denominators
produced by an appended ones-column in V), then a PARTIAL output projection
over its own 8 heads (contraction 512) for all 2048 queries. The host sums
the two partial projections of each batch (the tensor-parallel all-reduce,
done host-side because on-device collectives on this fabric run ~30 GB/s);
ATTN_COMBINE=rs switches to an on-device chunked pairwise ReduceScatter.

Single fused pipeline: attention steps t=(head, qh, j) emit S(t) on the PE
and exp(t) on ACT, with AV(t-1) lagged one step. The QKV projection is NOT a
separate phase: K/Q/V tile computations are interleaved as "filler" PE work
inside the attention step stream (the attention loop is ACT-bound, so the PE
slack absorbs them), sharing the PSUM s-ring. kT is stored zero-padded per
head on the full 128 partitions so every matmul in the kernel runs in the
same (128,128) PE tile mode — mode switches would expose every LDWEIGHTS.

Everything is bf16 into fp32 PSUM; partial projections are exchanged fp32.
"""
import os
import sys
import types

import numpy as np

import concourse.bass as bass
import concourse.mybir as mybir
import concourse.tile as tile
from concourse import bacc, bass_utils

DT = getattr(mybir.dt, os.environ.get("ATTN_DT_MAIN", "bfloat16"))
F32 = mybir.dt.float32
AF = mybir.ActivationFunctionType
OP = mybir.AluOpType

B, T, D = 4, 2048, 1024
H, HD = 16, 64
HG = 8          # heads per core
QH = T // 2     # query half
N_CORES = 8
SCALE = 1.0 / 8.0

ADT_NAME = os.environ.get("ATTN_DTYPE", "bfloat16")
ADT = getattr(mybir.dt, ADT_NAME)
COMBINE = os.environ.get("ATTN_COMBINE", "hostsum")

RG_PAIRS = [[0, 1], [2, 3], [4, 5], [6, 7]]


# ---------------------------------------------------------------------------
# environment glue
# ---------------------------------------------------------------------------

def _install_ntff_hook():
    if 'antenv.axon_hooks' in sys.modules:
        return
    try:
        from trn_agent_boot.trn_boot import _ntff_profile_via_ctypes
        hook = _ntff_profile_via_ctypes('/opt/axon/libaxon_pjrt.so')
    except Exception:
        hook = None
    mod = types.ModuleType('antenv.axon_hooks')
    mod.get_axon_ntff_profile_hook = lambda: hook
    mod.set_axon_ntff_profile_hook = lambda h: None
    sys.modules['antenv.axon_hooks'] = mod


def _run_spmd(nc, in_maps, trace=False):
    from concourse.bass_interp import get_hw_module
    bass_utils.upload_artifacts = lambda tmpdir: tmpdir
    if trace:
        _install_ntff_hook()
    old_m = nc.m
    nc.m = get_hw_module(nc.m)
    try:
        return bass_utils.run_bass_kernel_spmd(
            nc, in_maps, core_ids=list(range(N_CORES)),
            trace=trace, trace_cores=[0] if trace else None,
        )
    finally:
        nc.m = old_m


# ---------------------------------------------------------------------------
# kernel program
# ---------------------------------------------------------------------------

def build_nc(combine):
    nc = bacc.Bacc("TRN2", target_bir_lowering=False, debug=False,
                   enable_asserts=False, num_devices=N_CORES)
    xT = nc.dram_tensor("xT", [D, T], DT, kind="ExternalInput").ap()
    wqT = nc.dram_tensor("wqT", [D, 512], DT, kind="ExternalInput").ap()
    wkT = nc.dram_tensor("wkT", [D, 512], DT, kind="ExternalInput").ap()
    wvT = nc.dram_tensor("wvT", [D, 512], DT, kind="ExternalInput").ap()
    wpT = nc.dram_tensor("wpT", [512, D], DT, kind="ExternalInput").ap()
    mask = nc.dram_tensor("mask", [128, 128], ADT, kind="ExternalInput").ap()
    vone = nc.dram_tensor("vone", [128, 128], ADT, kind="ExternalInput").ap()
    bias = nc.dram_tensor("bias", [1, D], F32, kind="ExternalInput").ap()
    snum = nc.dram_tensor("snum", [32, QH], F32).ap()
    srec = nc.dram_tensor("srec", [32, QH], ADT).ap()
    if combine == "rs":
        rsin = [nc.dram_tensor(f"rsin{i}", [2, 128, D], F32).ap()
                for i in range(8)]
        yint = nc.dram_tensor("yint", [8, 128, D], F32).ap()
        yo = nc.dram_tensor("yo", [8, 128, D], F32, kind="ExternalOutput").ap()
    else:
        yo = nc.dram_tensor("yo", [T, D], F32, kind="ExternalOutput").ap()

    from contextlib import ExitStack
    with tile.TileContext(nc) as tc, ExitStack() as ctx:
        per = ctx.enter_context(tc.tile_pool(name="per", bufs=1))

        qT_sb = per.tile([128, 4, T], ADT, tag="qT")
        kT_sb = per.tile([128, 8, T], ADT, tag="kT")
        v_sb = per.tile([128, 16, HG, 65], ADT, tag="v")
        mask_sb = per.tile([128, 128], ADT, tag="mask")
        o_all = per.tile([128, 4, T], ADT, tag="oacc")
        wp_sb = per.tile([128, 4, D], DT, tag="wp")
        bias_bc = per.tile([128, D], F32, tag="bbc")

        nc.sync.dma_start(mask_sb[:], mask[:])
        # zero the dead half of each head's kT slot (gpsimd is otherwise idle)
        nc.gpsimd.memset(kT_sb[:], 0.0)

        xp = ctx.enter_context(tc.tile_pool(name="xph", bufs=16))
        wpo = ctx.enter_context(tc.tile_pool(name="wph", bufs=24))
        xT_r = xT.rearrange("(ko ki) t -> ki ko t", ki=128)

        # input DMAs, in consumption order: x halves for K(0), then weights
        xh = [[None] * 8 for _ in range(2)]
        for half in range(2):
            for kk in range(8):
                t = xp.tile([128, QH], DT, tag="xh", name=f"x{half}_{kk}")
                nc.sync.dma_start(t[:], xT_r[:, kk, half * QH:(half + 1) * QH])
                xh[half][kk] = t

        def load_w(wT, nm):
            parts = []
            wT_r = wT.rearrange("(ko ki) n -> ki ko n", ki=128)
            for kk in range(8):
                t = wpo.tile([128, 512], DT, tag="w", name=f"{nm}{kk}")
                nc.sync.dma_start(t[:], wT_r[:, kk])
                parts.append(t)
            return parts

        wk_sb = load_w(wkT, "wk")
        wq_sb = load_w(wqT, "wq")
        wv_sb = load_w(wvT, "wv")
        nc.sync.dma_start(v_sb[:, :, :, 64],
                          vone.rearrange("p (a b) -> p a b", a=16))
        nc.sync.dma_start(wp_sb[:],
                          wpT.rearrange("(ko ki) n -> ki ko n", ki=128))
        nc.sync.dma_start(bias_bc[:], bias[0][None, :].broadcast_to([128, D]))

        with ExitStack() as attn:
            sps = attn.enter_context(tc.tile_pool(name="sps", bufs=2, space="PSUM"))
            ops = attn.enter_context(tc.tile_pool(name="ops", bufs=2, space="PSUM"))
            es = attn.enter_context(tc.tile_pool(name="es", bufs=4))
            ev = attn.enter_context(tc.tile_pool(name="ev", bufs=2))
            nrm = attn.enter_context(tc.tile_pool(name="nrm", bufs=2))

            # ---- filler units: K/Q/V tile computations fed into PE slack --
            def emit_K(m, half):
                # kT for heads (2m, 2m+1), zero-padded layout
                pt = sps.tile([128, QH], F32, tag="s", name=f"ptk{m}_{half}")
                for nch in range(2):
                    sl = slice(nch * 512, (nch + 1) * 512)
                    for kk in range(8):
                        nc.tensor.matmul(
                            pt[:, sl],
                            lhsT=wk_sb[kk][:, m * 128:(m + 1) * 128],
                            rhs=xh[half][kk][:, sl],
                            start=(kk == 0), stop=(kk == 7))
                tsl = slice(half * QH, (half + 1) * QH)
                nc.vector.tensor_copy(kT_sb[0:64, 2 * m, tsl], pt[0:64, :])
                nc.vector.tensor_copy(kT_sb[64:128, 2 * m + 1, tsl],
                                      pt[64:128, :])

            def emit_Q(m, half):
                pt = sps.tile([128, QH], F32, tag="s", name=f"ptq{m}_{half}")
                for nch in range(2):
                    sl = slice(nch * 512, (nch + 1) * 512)
                    for kk in range(8):
                        nc.tensor.matmul(
                            pt[:, sl],
                            lhsT=wq_sb[kk][:, m * 128:(m + 1) * 128],
                            rhs=xh[half][kk][:, sl],
                            start=(kk == 0), stop=(kk == 7))
                nc.vector.tensor_copy(
                    qT_sb[:, m, half * QH:(half + 1) * QH], pt[:])

            def emit_V(m):
                # V' for key block m, all 8 heads
                pt = sps.tile([128, QH], F32, tag="s", name=f"ptv{m}")
                for kk in range(8):
                    nc.tensor.matmul(
                        pt[:, 0:512],
                        lhsT=xh[m // 8][kk][:, (m % 8) * 128:(m % 8 + 1) * 128],
                        rhs=wv_sb[kk][:],
                        start=(kk == 0), stop=(kk == 7))
                nc.vector.tensor_copy(
                    v_sb[:, m, :, 0:64],
                    pt[:, 0:512].rearrange("p (h d) -> p h d", h=HG))

            # ---- attention steps -----------------------------------------
            steps = []
            for h in range(HG):
                for qh in range(2):
                    for j in range(8 * qh + 8):
                        steps.append((h, qh, j))
            n_steps = len(steps)

            e_tiles = [None] * n_steps
            o_tiles = {}

            def emit_S_exp(t):
                h, qh, j = steps[t]
                qstart = max(QH * qh, 128 * j)
                n = QH * (qh + 1) - qstart
                sub = h // 2
                s_ps = sps.tile([128, QH], F32, tag="s", name=f"s{t}")
                for c in range(0, n, 512):
                    cn = min(512, n - c)
                    nc.tensor.matmul(
                        s_ps[:, c:c + cn],
                        lhsT=kT_sb[:, h, j * 128:(j + 1) * 128],
                        rhs=qT_sb[:, sub, qstart + c:qstart + c + cn],
                        start=True, stop=True)
                e_sb = es.tile([128, QH], ADT, tag="e", name=f"e{t}")
                nc.scalar.activation(e_sb[:, 0:n], s_ps[:, 0:n], AF.Exp,
                                     scale=SCALE)
                if j >= 8 * qh:
                    nc.gpsimd.tensor_tensor(
                        e_sb[:, 0:128], e_sb[:, 0:128], mask_sb[:], OP.mult)
                e_tiles[t] = (e_sb, n, qstart - QH * qh)

            def emit_AV(t):
                h, qh, j = steps[t]
                e_sb, n, coff = e_tiles[t]
                if j == 0:
                    o_tiles[(h, qh)] = ops.tile([65, QH], F32, tag="o",
                                                name=f"o{h}_{qh}")
                o_ps = o_tiles[(h, qh)]
                jmax = 8 * qh + 8
                c0 = coff
                while c0 < QH:
                    hi = min(QH, (c0 // 512 + 1) * 512)
                    nc.tensor.matmul(
                        o_ps[:, c0:hi],
                        lhsT=v_sb[:, j, h, :],
                        rhs=e_sb[:, c0 - coff:hi - coff],
                        start=(j == 0), stop=(j == jmax - 1),
                        skip_group_check=True)
                    c0 = hi
                e_tiles[t] = None

            def emit_evict(h, qh):
                o_ps = o_tiles.pop((h, qh))
                pbase = 64 * (h % 2)
                sub = h // 2
                i = 2 * h + qh
                dtile = ev.tile([1, QH], F32, tag="dn", name=f"dn{h}_{qh}")
                nc.vector.tensor_copy(dtile[:], o_ps[64:65, :])
                nc.sync.dma_start(snum[i:i + 1, :], dtile[:])
                if pbase == 0:
                    nc.vector.tensor_copy(
                        o_all[0:64, sub, QH * qh:QH * (qh + 1)], o_ps[0:64, :])
                else:
                    tmp = ev.tile([64, QH], ADT, tag="ev", name=f"ev{h}_{qh}")
                    nc.vector.tensor_copy(tmp[:], o_ps[0:64, :])
                    nc.sync.dma_start(
                        o_all[64:128, sub, QH * qh:QH * (qh + 1)], tmp[:])
                st64 = nrm.tile([64, QH // 64], F32, tag="sp")
                nc.sync.dma_start(st64[:], snum[i].rearrange("(p f) -> p f", p=64))
                nc.vector.reciprocal(st64[:], st64[:])
                st64b = nrm.tile([64, QH // 64], ADT, tag="spb")
                nc.vector.tensor_copy(st64b[:], st64[:])
                nc.sync.dma_start(srec[i].rearrange("(p f) -> p f", p=64), st64b[:])
                bc = nrm.tile([128, QH], ADT, tag="bc")
                nc.sync.dma_start(bc[:], srec[i][None, :].broadcast_to([128, QH]))
                sl_ap = o_all[pbase:pbase + 64, sub, QH * qh:QH * (qh + 1)]
                nc.vector.tensor_tensor(sl_ap, sl_ap, bc[pbase:pbase + 64, :],
                                        OP.mult)

            # ---- filler schedule: emission position -> list of closures --
            fillers = {}

            def add_filler(pos, fn, *args):
                fillers.setdefault(pos, []).append((fn, args))

            # AV is lagged TWO steps behind S/exp so it never waits on ACT
            # and its ldweights hides under the S streams. V(j) therefore
            # has until emission j+2.
            for j in range(8):
                add_filler(j + 1, emit_V, j)            # AV(h0,qh0,j) at j+2
            for idx, j in enumerate(range(8, 16)):
                add_filler(9 + idx, emit_V, j)          # AV(h0,qh1,j) at j+10
            add_filler(2, emit_Q, 0, 1)                 # h0/h1 qh1 from step 8
            add_filler(4, emit_K, 0, 1)                 # h0 qh1 j>=8 at step 16
            for p, base in ((1, 28), (2, 76), (3, 124)):
                add_filler(base, emit_K, p, 0)          # by step 48p
                add_filler(base + 4, emit_Q, p, 0)
                add_filler(base + 8, emit_Q, p, 1)      # by step 48p+8
                add_filler(base + 12, emit_K, p, 1)     # by step 48p+16
            # ---- fused emission ------------------------------------------
            emit_K(0, 0)
            emit_Q(0, 0)
            for t in range(n_steps):
                emit_S_exp(t)
                for fn, args in fillers.get(t, []):
                    fn(*args)
                if t > 1:
                    emit_AV(t - 2)
                    h0, qh0, j0 = steps[t - 2]
                    if j0 == 8 * qh0 + 7:
                        emit_evict(h0, qh0)
            for t in (n_steps - 2, n_steps - 1):
                emit_AV(t)
                h0, qh0, j0 = steps[t]
                if j0 == 8 * qh0 + 7:
                    emit_evict(h0, qh0)

        # ---------------- partial projection -------------------------------
        yop = ctx.enter_context(tc.tile_pool(name="yop", bufs=3))
        pps = ctx.enter_context(tc.tile_pool(name="pps", bufs=2, space="PSUM"))
        for i in range(8):
            for g in range(2):
                m = 8 * g + i
                yp = pps.tile([128, D], F32, tag="yp")
                for nch in range(2):
                    sl = slice(nch * 512, (nch + 1) * 512)
                    for kk in range(4):
                        nc.tensor.matmul(
                            yp[:, sl],
                            lhsT=o_all[:, kk, m * 128:(m + 1) * 128],
                            rhs=wp_sb[:, kk, sl],
                            start=(kk == 0), stop=(kk == 3))
                y_sb = yop.tile([128, D], F32, tag="y")
                nc.vector.tensor_tensor(y_sb[:], yp[:], bias_bc[:], OP.add)
                if combine == "rs":
                    nc.sync.dma_start(rsin[i][g], y_sb[:])
                else:
                    nc.sync.dma_start(yo[m * 128:(m + 1) * 128, :], y_sb[:])
            if combine == "rs":
                nc.gpsimd.collective_compute(
                    "ReduceScatter", OP.add,
                    replica_groups=RG_PAIRS,
                    ins=[rsin[i][:]], outs=[yint[i]],
                )
                nc.sync.dma_start(yo[i], yint[i])

    nc.compile()
    return nc


# ---------------------------------------------------------------------------
# host-side sharding + entry point
# ---------------------------------------------------------------------------

_NC_CACHE = {}


def _get_nc(combine):
    if combine not in _NC_CACHE:
        _NC_CACHE[combine] = build_nc(combine)
    return _NC_CACHE[combine]


def _make_in_maps(x, Wq, Wk, Wv, Wp, bp):
    x = np.asarray(x, dtype=np.float32)
    Wq = np.asarray(Wq, dtype=np.float32)
    Wk = np.asarray(Wk, dtype=np.float32)
    Wv = np.asarray(Wv, dtype=np.float32)
    Wp = np.asarray(Wp, dtype=np.float32)
    bp = np.asarray(bp, dtype=np.float32)

    adt_np = mybir.dt.np(ADT)
    dt_np = mybir.dt.np(DT)
    mask = np.zeros((128, 128), dtype=np.float32)
    k_idx = np.arange(128)[:, None]
    q_idx = np.arange(128)[None, :]
    mask[q_idx >= k_idx] = 1.0
    mask = mask.astype(adt_np)

    xTs = [np.ascontiguousarray(x[b].T) for b in range(B)]
    in_maps = []
    for c in range(N_CORES):
        b, g = c // 2, c % 2
        rows = slice(512 * g, 512 * (g + 1))
        m = {
            "xT": xTs[b].astype(dt_np),
            "wqT": np.ascontiguousarray(Wq[rows, :].T).astype(dt_np),
            "wkT": np.ascontiguousarray(Wk[rows, :].T).astype(dt_np),
            "wvT": np.ascontiguousarray(Wv[rows, :].T).astype(dt_np),
            "wpT": np.ascontiguousarray(Wp[:, rows].T).astype(dt_np),
            "mask": mask,
            "vone": np.ones((128, 128), dtype=adt_np),
            "bias": (bp if g == 0 else np.zeros_like(bp)).reshape(1, D),
        }
        in_maps.append(m)
    return in_maps


def kernel(x, Wq, Wk, Wv, Wp, bp, _trace=False):
    combine = COMBINE
    nc = _get_nc(combine)
    in_maps = _make_in_maps(x, Wq, Wk, Wv, Wp, bp)
    res = _run_spmd(nc, in_maps, trace=_trace)
    out = np.empty((B, T, D), dtype=np.float32)
    for b in range(B):
        ya = res.results[2 * b]["yo"]
        yb = res.results[2 * b + 1]["yo"]
        if combine == "rs":
            out[b, 0:QH] = ya.reshape(QH, D)
            out[b, QH:T] = yb.reshape(QH, D)
        else:
            out[b] = ya + yb
    if _trace:
        kernel.last_results = res
    return out
